# revision 24
# baseline (speedup 1.0000x reference)
"""Multi-head attention (B=2, S=2048, D=1024, H=16) on 8 Trainium2 NeuronCores.

Sharding: core c handles (batch b=c//4, head-group g=c%4 of 4 heads) for ALL
2048 queries — head/tensor parallel instead of the old query-parallel split.
 - Q/K/V projections only cover the core's 256 features (4x less PE work than
   replicating K/V per batch; no collectives needed).
 - Attention (4 heads x 2048 queries x 2048 keys):
   scores^T = K_h^T-pair @ Q_h^T as K=64-contraction matmuls in alternating
   PE row groups (two heads run concurrently in the array),
   exp on ACT at FD=1024, attnT = [V_h|1]^T @ E with 65-col stationaries
   (ones column gives the softmax denominator Z in psum row 64).
 - Normalize uses the fast approximate reciprocal custom DVE op.
 - Output projection contracts only the local 256 features -> each core emits
   a PARTIAL output [2048, 1024] bf16; the host sums the 4 partials per batch
   and adds the (V-bias-folded) output bias.
"""

import numpy as np
import ml_dtypes

import concourse.bass as bass
import concourse.mybir as mybir
import concourse.tile as tile
from concourse import bacc
from concourse.bass_utils import run_bass_kernel_spmd

BF16 = mybir.dt.bfloat16
F32 = mybir.dt.float32
AF = mybir.ActivationFunctionType

B, S, D = 2, 2048, 1024
H, HD = 16, 64
N_CORES = 8
G = 4              # head-groups per batch (cores per batch)
HL = H // G        # heads per core (4)
FL = HL * HD       # local projected features (256)
P = 128
DCH = D // P       # 8 contraction chunks
NKK = S // P       # 16 key chunks
QC = 512           # query block
NQC = S // QC      # 4
VW = HL * (HD + 1) + HD  # packed [V|1] width + 64 pad so 65h+65 slices stay
                         # inside one dense region (pad cols memset to 0)


DEBUG_DUMP = False


def build_program():
    nc = bacc.Bacc("TRN2", target_bir_lowering=False, debug=False,
                   num_devices=N_CORES)

    xT = nc.dram_tensor("xT", [D, S], BF16, kind="ExternalInput")
    wqT = nc.dram_tensor("wqT", [D, FL], BF16, kind="ExternalInput")
    wkT = nc.dram_tensor("wkT", [D, FL], BF16, kind="ExternalInput")
    wvT = nc.dram_tensor("wvT", [D, FL], BF16, kind="ExternalInput")
    woT = nc.dram_tensor("woT", [FL, D], BF16, kind="ExternalInput")
    bqk = nc.dram_tensor("bqk", [P, 4], F32, kind="ExternalInput")
    out = nc.dram_tensor("out", [S, D], BF16, kind="ExternalOutput")
    dbg = {}
    if DEBUG_DUMP:
        for nm, shape, dt in (
                ("dbg_kt0", [P, S], BF16), ("dbg_qp0", [P, S], BF16),
                ("dbg_vt0", [P, VW], BF16), ("dbg_e", [P, 2 * QC], BF16),
                ("dbg_atsb", [HD + 1, QC], F32), ("dbg_rz", [1, QC], F32),
                ("dbg_rzb", [HD, QC], F32), ("dbg_asb0", [P, S], BF16)):
            dbg[nm] = nc.dram_tensor(nm, shape, dt, kind="ExternalOutput")

    with tile.TileContext(nc) as tc:
        _build(nc, tc, xT, wqT, wkT, wvT, woT, bqk, out, dbg)
    nc.compile()
    return nc


def _build(nc, tc, xT, wqT, wkT, wvT, woT, bqk, out, dbg=()):
    from contextlib import ExitStack

    ctx = ExitStack()
    consts = ctx.enter_context(tc.tile_pool(name="consts", bufs=1))
    bqk_sb = consts.tile([P, 4], F32, name="bqk_sb")
    nc.sync.dma_start(bqk_sb[:], bqk.ap())

    # ---- resident input tiles: first halves on sync, rest on gpsimd ----
    xt_pool = ctx.enter_context(tc.tile_pool(name="xt", bufs=1))
    xt = [xt_pool.tile([P, S], BF16, name=f"xt{i}") for i in range(DCH)]
    for ch in (0, 1):
        for i in range(DCH):
            nc.sync.dma_start(xt[i][:, QC * ch:QC * (ch + 1)],
                              xT.ap()[P * i:P * (i + 1),
                                      QC * ch:QC * (ch + 1)])

    # ---- weights (gpsimd queue, in consumption order), then xt tails ----
    w_pool = ctx.enter_context(tc.tile_pool(name="w", bufs=1))
    wk, wq, wv = [], [], []
    for nm, dram, lst in (("wk", wkT, wk), ("wq", wqT, wq), ("wv", wvT, wv)):
        for d in range(DCH):
            t = w_pool.tile([P, FL], BF16, name=f"{nm}{d}")
            nc.gpsimd.dma_start(t[:], dram.ap()[P * d:P * (d + 1), :])
            lst.append(t)
    for ch in (2, 3):
        for i in range(DCH):
            nc.gpsimd.dma_start(xt[i][:, QC * ch:QC * (ch + 1)],
                                xT.ap()[P * i:P * (i + 1),
                                        QC * ch:QC * (ch + 1)])
    wo = []
    for p_ in range(2):
        t = w_pool.tile([P, D], BF16, name=f"wo{p_}")
        nc.gpsimd.dma_start(t[:], woT.ap()[P * p_:P * (p_ + 1), :])
        wo.append(t)

    # ---- persistent compute tiles ----
    kv_pool = ctx.enter_context(tc.tile_pool(name="kv", bufs=1))
    kt = [kv_pool.tile([P, S], BF16, name=f"kt{t}") for t in range(2)]
    qz = [kv_pool.tile([P, S], BF16, name=f"qz{h}") for h in range(HL)]
    for h in range(4):
        off = HD * ((h + 1) % 2)
        nc.vector.memset(qz[h][off:off + HD, :], 0.0)
    vt = [kv_pool.tile([P, VW], BF16, name=f"vt{g}") for g in range(NKK)]
    for g in range(NKK):
        v3 = vt[g][:, 0:HL * (HD + 1)].rearrange("p (h c) -> p h c", c=HD + 1)
        nc.vector.memset(v3[:, :, HD:HD + 1], 1.0)
        nc.vector.memset(vt[g][:, HL * (HD + 1):VW], 0.0)
    attn_sb = [kv_pool.tile([P, S], BF16, name=f"asb{t}") for t in range(2)]
    osb = [kv_pool.tile([P, D], BF16, name=f"osb{st}")
           for st in range(S // P)]

    # ---- one flat instruction stream: projections, attention, output ----
    # PSUM budget (8 banks): sc 2x[128,1024] = 4, at 2x[65,512] = 2,
    # op/vproj 2x[128,512] = 2. K/Q projection units borrow sc tiles;
    # V projection borrows op tiles, so every phase weaves into the stream.
    HS = S // 2
    small_pool = ctx.enter_context(tc.tile_pool(name="small", bufs=8))

    with tc.tile_pool(name="sc_ps", bufs=2, space="PSUM") as sc_ps, \
         tc.tile_pool(name="at_ps", bufs=3, space="PSUM") as at_ps, \
         tc.tile_pool(name="op_ps", bufs=1, space="PSUM") as op_ps, \
         tc.tile_pool(name="e_sb", bufs=6) as e_pool:

        def proj_unit(which, t, sh):
            ps = sc_ps.tile([P, HS], F32, name=f"ps{which}{t}_{sh}", tag="sc")
            w = wk if which == "k" else wq
            for d in range(DCH):
                for sch in range(2):
                    co = HS * sh + QC * sch
                    nc.tensor.matmul(ps[:, QC * sch:QC * (sch + 1)],
                                     w[d][:, P * t:P * (t + 1)],
                                     xt[d][:, co:co + QC],
                                     start=(d == 0), stop=(d == DCH - 1))
            if which == "k":
                nc.vector.tensor_scalar_add(kt[t][:, HS * sh:HS * (sh + 1)],
                                            ps[:], bqk_sb[:, 2 + t:3 + t])
                if dbg and t == 0 and sh == 1:
                    nc.sync.dma_start(dbg["dbg_kt0"].ap(), kt[0][:])
            else:
                nc.vector.tensor_scalar_add(
                    qz[2 * t][0:HD, HS * sh:HS * (sh + 1)],
                    ps[0:HD, :], bqk_sb[0:HD, t:t + 1])
                nc.vector.tensor_scalar_add(
                    qz[2 * t + 1][HD:P, HS * sh:HS * (sh + 1)],
                    ps[HD:P, :], bqk_sb[HD:P, t:t + 1])

        def vproj_pair(pr):
            ps = op_ps.tile([P, 2 * FL], F32, name=f"psv{pr}", tag="op")
            for half in range(2):
                st = 2 * pr + half
                for d in range(DCH):
                    nc.tensor.matmul(ps[:, FL * half:FL * (half + 1)],
                                     xt[d][:, P * st:P * (st + 1)], wv[d][:],
                                     start=(d == 0), stop=(d == DCH - 1))
            for half in range(2):
                st = 2 * pr + half
                v3 = vt[st][:, 0:HL * (HD + 1)].rearrange(
                    "p (h c) -> p h c", c=HD + 1)
                nc.vector.tensor_copy(
                    v3[:, :, 0:HD],
                    ps[:, FL * half:FL * (half + 1)].rearrange(
                        "p (h dd) -> p h dd", dd=HD))
            if dbg and pr == 0:
                nc.sync.dma_start(dbg["dbg_vt0"].ap(), vt[0][:])

        def run_op(st, act_evac=False):
            for eb in range(2):
                op = op_ps.tile([P, QC], F32, name=f"op{st}_{eb}", tag="op")
                for p_ in range(2):
                    nc.tensor.matmul(op[:],
                                     attn_sb[p_][:, P * st:P * (st + 1)],
                                     wo[p_][:, QC * eb:QC * (eb + 1)],
                                     start=(p_ == 0), stop=(p_ == 1))
                if act_evac and eb == 0:
                    nc.scalar.activation(osb[st][:, 0:QC], op[:], AF.Identity)
                else:
                    nc.vector.tensor_copy(osb[st][:, QC * eb:QC * (eb + 1)],
                                          op[:])
            nc.sync.dma_start(out.ap()[P * st:P * (st + 1), :], osb[st][:])

        def normalize(qc, t, par, at):
            last = (qc == NQC - 1 and t == 1)
            atsb = small_pool.tile([HD, QC], F32,
                                   name=f"atsb{qc}_{t}_{par}", tag="atsb")
            zr = small_pool.tile([1, QC], F32, name=f"zr{qc}_{t}_{par}",
                                 tag="zr")
            if last:
                nc.scalar.activation(atsb[:], at[0:HD, :], AF.Identity)
                nc.scalar.activation(zr[:], at[HD:HD + 1, :], AF.Identity)
            else:
                nc.vector.tensor_copy(atsb[:], at[0:HD, :])
                nc.vector.tensor_copy(zr[:], at[HD:HD + 1, :])
            rz = small_pool.tile([1, QC], F32, name=f"rz{qc}_{t}_{par}",
                                 tag="rz")
            nc.vector.reciprocal_approx_fast(rz[:], zr[:])
            rzb = small_pool.tile([HD, QC], F32, name=f"rzb{qc}_{t}_{par}",
                                  tag="rzb")
            nc.gpsimd.partition_broadcast(rzb[:], rz[:])
            nc.vector.tensor_mul(
                attn_sb[t][HD * par:HD * (par + 1), QC * qc:QC * (qc + 1)],
                atsb[:], rzb[:])
            if dbg and qc == 0 and t == 0 and par == 0:
                nc.sync.dma_start(dbg["dbg_atsb"].ap(), atsb[:])
                nc.sync.dma_start(dbg["dbg_rz"].ap(), rz[:])
                nc.sync.dma_start(dbg["dbg_rzb"].ap(), rzb[:])

        units = [(qc, t, kk, par) for qc in range(NQC) for t in range(2)
                 for kk in range(NKK) for par in range(2)]
        # interjected work, keyed by unit index (deadline-driven):
        # kt[1]/qp[1] before unit 32, all vt pairs before their PV drains,
        # second-half q tiles before unit 128.
        interject = {
            0: [("k", 0, 1)], 2: [("v", 1)], 6: [("v", 2)],
            10: [("k", 1, 0)], 14: [("v", 3)], 18: [("k", 1, 1)],
            22: [("v", 4)], 26: [("q", 1, 0)], 30: [("v", 5)],
            34: [("v", 6)], 38: [("v", 7)], 48: [("q", 0, 1)],
            56: [("q", 1, 1)],
        }
        pend = []            # (qc, t, kk, par, group, e_tile, col_off)
        at_tiles = {}
        vpair_unit = {0: -100}
        op_queue = []
        group_idx = 0
        sc_cur, cur = None, []

        def drain_one(u, force=False):
            if not pend:
                return False
            qc, t, kk, par, g, et, off = pend[0]
            if not force:
                if g >= group_idx - (2 if kk == 0 else 0):
                    return False
                if vpair_unit.get(kk // 2, 10 ** 9) > u - 3:
                    return False
            pend.pop(0)
            key = (qc, t, par)
            if key not in at_tiles:
                at_tiles[key] = at_ps.tile([HD + 1, QC], F32,
                                           name=f"at{qc}_{t}_{par}", tag="at")
            h = 2 * t + par
            nc.tensor.matmul(at_tiles[key][:], vt[kk][:, 65 * h:65 * h + 65],
                             et[:, off:off + QC],
                             start=(kk == 0), stop=(kk == NKK - 1))
            if kk == NKK - 1:
                normalize(qc, t, par, at_tiles.pop(key))
                if t == 1 and par == 1:
                    op_queue.extend((st, u + 16) for st in
                                    range(4 * qc, 4 * qc + 4))
            return True

        def proj_unit_fast(which, t):
            # sch-outer with split casts: the first 512 columns complete as
            # soon as the first DMA'd chunk of every xt tile lands
            ps = sc_ps.tile([P, HS], F32, name=f"psf{which}{t}", tag="sc")
            w = wk if which == "k" else wq
            for sch in range(2):
                for d in range(DCH):
                    nc.tensor.matmul(ps[:, QC * sch:QC * (sch + 1)],
                                     w[d][:, P * t:P * (t + 1)],
                                     xt[d][:, QC * sch:QC * (sch + 1)],
                                     start=(d == 0), stop=(d == DCH - 1))
                if which == "k":
                    nc.scalar.activation(
                        kt[t][:, QC * sch:QC * (sch + 1)],
                        ps[:, QC * sch:QC * (sch + 1)], AF.Identity,
                        bias=bqk_sb[:, 2 + t:3 + t])
                else:
                    nc.scalar.activation(
                        qz[2 * t][0:HD, QC * sch:QC * (sch + 1)],
                        ps[0:HD, QC * sch:QC * (sch + 1)], AF.Identity,
                        bias=bqk_sb[0:HD, t:t + 1])
                    nc.scalar.activation(
                        qz[2 * t + 1][HD:P, QC * sch:QC * (sch + 1)],
                        ps[HD:P, QC * sch:QC * (sch + 1)], AF.Identity,
                        bias=bqk_sb[HD:P, t:t + 1])

        # preamble: the minimum needed before the first score matmul
        proj_unit_fast("k", 0)
        proj_unit_fast("q", 0)
        vproj_pair(0)

        for u, (qc, t, kk, par) in enumerate(units):
            for ij in interject.get(u, []):
                if ij[0] == "v":
                    vproj_pair(ij[1])
                    vpair_unit[ij[1]] = u
                else:
                    proj_unit(*ij)
            if sc_cur is None:
                sc_cur = sc_ps.tile([P, 2 * QC], F32, name=f"sc{u}", tag="sc")
                cur = []
            nc.tensor.matmul(sc_cur[:, QC * len(cur):QC * (len(cur) + 1)],
                             kt[t][:, P * kk:P * (kk + 1)],
                             qz[2 * t + par][:, QC * qc:QC * (qc + 1)],
                             start=True, stop=True)
            cur.append((qc, t, kk, par))
            if len(cur) == 2:
                e = e_pool.tile([P, 2 * QC], BF16, name=f"e{u}", tag="e")
                nc.scalar.activation(e[:], sc_cur[:], AF.Exp)
                if dbg and u == 1:
                    nc.sync.dma_start(dbg["dbg_e"].ap(), e[:])
                for j, cu in enumerate(cur):
                    pend.append((*cu, group_idx, e, QC * j))
                group_idx += 1
                sc_cur = None
            drained = 0
            while drained < 3 and drain_one(u):
                drained += 1
            if u % 8 == 5 and op_queue and op_queue[0][1] <= u:
                run_op(op_queue.pop(0)[0])
        while pend:
            drain_one(10 ** 9, force=True)
        while op_queue:
            run_op(op_queue.pop(0)[0], act_evac=True)
        if dbg:
            nc.sync.dma_start(dbg["dbg_asb0"].ap(), attn_sb[0][:])

    ctx.close()


_CACHE = {}


def _get_program():
    if "nc" not in _CACHE:
        _CACHE["nc"] = build_program()
    return _CACHE["nc"]


def prep_inputs(input_tensor, qkv_weight, qkv_bias, out_weight, out_bias):
    """Host-side shard + transpose + cast. Returns in_maps for 8 cores."""
    x = np.asarray(input_tensor, np.float32)
    wqkv = np.asarray(qkv_weight, np.float32).copy()
    bqkv = np.asarray(qkv_bias, np.float32).copy()
    wout = np.asarray(out_weight, np.float32)
    scale = 1.0 / np.sqrt(np.float32(HD))
    wqkv[:D] *= scale
    bqkv[:D] *= scale
    bf = ml_dtypes.bfloat16
    woutT = np.ascontiguousarray(wout.T)
    xTb = [np.ascontiguousarray(x[b].T).astype(bf) for b in range(B)]
    in_maps = []
    for c in range(N_CORES):
        b, g = c // G, c % G
        lo = FL * g
        wqT = np.ascontiguousarray(wqkv[lo:lo + FL, :].T).astype(bf)
        wkT = np.ascontiguousarray(wqkv[D + lo:D + lo + FL, :].T).astype(bf)
        wvT = np.ascontiguousarray(
            wqkv[2 * D + lo:2 * D + lo + FL, :].T).astype(bf)
        woTg = np.ascontiguousarray(woutT[lo:lo + FL, :]).astype(bf)
        bq = bqkv[lo:lo + FL].reshape(2, P).T
        bk = bqkv[D + lo:D + lo + FL].reshape(2, P).T
        bqk = np.ascontiguousarray(
            np.concatenate([bq, bk], 1)).astype(np.float32)
        in_maps.append({"xT": xTb[b], "wqT": wqT, "wkT": wkT, "wvT": wvT,
                       "woT": woTg, "bqk": bqk})
    return in_maps


def assemble(outs, qkv_bias, out_weight, out_bias):
    """Sum the per-core partials and add the (V-bias-folded) output bias."""
    bqkv = np.asarray(qkv_bias, np.float32)
    wout = np.asarray(out_weight, np.float32)
    bout_eff = np.asarray(out_bias, np.float32) + wout @ bqkv[2 * D:]
    full = np.empty((B, S, D), np.float32)
    for b in range(B):
        acc = bout_eff[None, :].astype(np.float32).repeat(S, 0)
        for g in range(G):
            acc += np.asarray(outs[b * G + g], np.float32)
        full[b] = acc
    return full


def kernel(input_tensor, qkv_weight, qkv_bias, out_weight, out_bias,
           **run_kwargs):
    nc = _get_program()
    in_maps = prep_inputs(input_tensor, qkv_weight, qkv_bias, out_weight,
                          out_bias)
    res = run_bass_kernel_spmd(nc, in_maps, core_ids=list(range(N_CORES)),
                               **run_kwargs)
    full = assemble([res.results[c]["out"] for c in range(N_CORES)],
                    qkv_bias, out_weight, out_bias)
    if run_kwargs:
        kernel.last_results = res
    return full


# revision 26
# speedup vs baseline: 1.0192x; 1.0192x over previous
"""Multi-head attention (B=2, S=2048, D=1024, H=16) on 8 Trainium2 NeuronCores.

Sharding: core c handles (batch b=c//4, head-group g=c%4 of 4 heads) for ALL
2048 queries — head/tensor parallel instead of the old query-parallel split.
 - Q/K/V projections only cover the core's 256 features (4x less PE work than
   replicating K/V per batch; no collectives needed).
 - Attention (4 heads x 2048 queries x 2048 keys):
   scores^T = K_h^T-pair @ Q_h^T as K=64-contraction matmuls in alternating
   PE row groups (two heads run concurrently in the array),
   exp on ACT at FD=1024, attnT = [V_h|1]^T @ E with 65-col stationaries
   (ones column gives the softmax denominator Z in psum row 64).
 - Normalize uses the fast approximate reciprocal custom DVE op.
 - Output projection contracts only the local 256 features -> each core emits
   a PARTIAL output [2048, 1024] bf16; the host sums the 4 partials per batch
   and adds the (V-bias-folded) output bias.
"""

import numpy as np
import ml_dtypes

import concourse.bass as bass
import concourse.mybir as mybir
import concourse.tile as tile
from concourse import bacc
from concourse.bass_utils import run_bass_kernel_spmd

BF16 = mybir.dt.bfloat16
F32 = mybir.dt.float32
AF = mybir.ActivationFunctionType

B, S, D = 2, 2048, 1024
H, HD = 16, 64
N_CORES = 8
G = 4              # head-groups per batch (cores per batch)
HL = H // G        # heads per core (4)
FL = HL * HD       # local projected features (256)
P = 128
DCH = D // P       # 8 contraction chunks
NKK = S // P       # 16 key chunks
QC = 512           # query block
NQC = S // QC      # 4
VW = HL * (HD + 1) + HD  # packed [V|1] width + 64 pad so 65h+65 slices stay
                         # inside one dense region (pad cols memset to 0)


DEBUG_DUMP = False


def build_program():
    nc = bacc.Bacc("TRN2", target_bir_lowering=False, debug=False,
                   num_devices=N_CORES)

    xT = nc.dram_tensor("xT", [D, S], BF16, kind="ExternalInput")
    wqT = nc.dram_tensor("wqT", [D, FL], BF16, kind="ExternalInput")
    wkT = nc.dram_tensor("wkT", [D, FL], BF16, kind="ExternalInput")
    wvT = nc.dram_tensor("wvT", [D, FL], BF16, kind="ExternalInput")
    woT = nc.dram_tensor("woT", [FL, D], BF16, kind="ExternalInput")
    bqk = nc.dram_tensor("bqk", [P, 4], F32, kind="ExternalInput")
    out = nc.dram_tensor("out", [S, D], BF16, kind="ExternalOutput")
    dbg = {}
    if DEBUG_DUMP:
        for nm, shape, dt in (
                ("dbg_kt0", [P, S], BF16), ("dbg_qp0", [P, S], BF16),
                ("dbg_vt0", [P, VW], BF16), ("dbg_e", [P, 2 * QC], BF16),
                ("dbg_atsb", [HD + 1, QC], F32), ("dbg_rz", [1, QC], F32),
                ("dbg_rzb", [HD, QC], F32), ("dbg_asb0", [P, S], BF16)):
            dbg[nm] = nc.dram_tensor(nm, shape, dt, kind="ExternalOutput")

    with tile.TileContext(nc) as tc:
        _build(nc, tc, xT, wqT, wkT, wvT, woT, bqk, out, dbg)
    nc.compile()
    return nc


def _build(nc, tc, xT, wqT, wkT, wvT, woT, bqk, out, dbg=()):
    from contextlib import ExitStack

    ctx = ExitStack()
    consts = ctx.enter_context(tc.tile_pool(name="consts", bufs=1))
    bqk_sb = consts.tile([P, 4], F32, name="bqk_sb")
    nc.sync.dma_start(bqk_sb[:], bqk.ap())

    # ---- resident input tiles: first halves on sync, rest on gpsimd ----
    xt_pool = ctx.enter_context(tc.tile_pool(name="xt", bufs=1))
    xt = [xt_pool.tile([P, S], BF16, name=f"xt{i}") for i in range(DCH)]
    for ch in (0, 1):
        for i in range(DCH):
            nc.sync.dma_start(xt[i][:, QC * ch:QC * (ch + 1)],
                              xT.ap()[P * i:P * (i + 1),
                                      QC * ch:QC * (ch + 1)])

    # ---- weights (gpsimd queue, in consumption order), then xt tails ----
    w_pool = ctx.enter_context(tc.tile_pool(name="w", bufs=1))
    wk, wq, wv = [], [], []
    for nm, dram, lst in (("wk", wkT, wk), ("wq", wqT, wq), ("wv", wvT, wv)):
        for d in range(DCH):
            t = w_pool.tile([P, FL], BF16, name=f"{nm}{d}")
            nc.gpsimd.dma_start(t[:], dram.ap()[P * d:P * (d + 1), :])
            lst.append(t)
    for ch in (2, 3):
        for i in range(DCH):
            nc.gpsimd.dma_start(xt[i][:, QC * ch:QC * (ch + 1)],
                                xT.ap()[P * i:P * (i + 1),
                                        QC * ch:QC * (ch + 1)])
    wo = []
    for p_ in range(2):
        t = w_pool.tile([P, D], BF16, name=f"wo{p_}")
        nc.gpsimd.dma_start(t[:], woT.ap()[P * p_:P * (p_ + 1), :])
        wo.append(t)

    # ---- persistent compute tiles ----
    kv_pool = ctx.enter_context(tc.tile_pool(name="kv", bufs=1))
    kt = [kv_pool.tile([P, S], BF16, name=f"kt{t}") for t in range(2)]
    qz = [kv_pool.tile([P, S], BF16, name=f"qz{h}") for h in range(HL)]
    for h in range(4):
        off = HD * ((h + 1) % 2)
        nc.vector.memset(qz[h][off:off + HD, :], 0.0)
    vt = [kv_pool.tile([P, VW], BF16, name=f"vt{g}") for g in range(NKK)]
    for g in range(NKK):
        v3 = vt[g][:, 0:HL * (HD + 1)].rearrange("p (h c) -> p h c", c=HD + 1)
        nc.vector.memset(v3[:, :, HD:HD + 1], 1.0)
        nc.vector.memset(vt[g][:, HL * (HD + 1):VW], 0.0)
    attn_sb = [kv_pool.tile([P, S], BF16, name=f"asb{t}") for t in range(2)]
    osb = [kv_pool.tile([P, D], BF16, name=f"osb{st}")
           for st in range(S // P)]

    # ---- one flat instruction stream: projections, attention, output ----
    # PSUM budget (8 banks): sc 2x[128,1024] = 4, at 2x[65,512] = 2,
    # op/vproj 2x[128,512] = 2. K/Q projection units borrow sc tiles;
    # V projection borrows op tiles, so every phase weaves into the stream.
    HS = S // 2
    small_pool = ctx.enter_context(tc.tile_pool(name="small", bufs=8))

    with tc.tile_pool(name="sc_ps", bufs=2, space="PSUM") as sc_ps, \
         tc.tile_pool(name="at_ps", bufs=2, space="PSUM") as at_ps, \
         tc.tile_pool(name="op_ps", bufs=2, space="PSUM") as op_ps, \
         tc.tile_pool(name="e_sb", bufs=8) as e_pool:

        def proj_unit(which, t, sh):
            ps = sc_ps.tile([P, HS], F32, name=f"ps{which}{t}_{sh}", tag="sc")
            w = wk if which == "k" else wq
            for d in range(DCH):
                for sch in range(2):
                    co = HS * sh + QC * sch
                    nc.tensor.matmul(ps[:, QC * sch:QC * (sch + 1)],
                                     w[d][:, P * t:P * (t + 1)],
                                     xt[d][:, co:co + QC],
                                     start=(d == 0), stop=(d == DCH - 1))
            if which == "k":
                nc.vector.tensor_scalar_add(kt[t][:, HS * sh:HS * (sh + 1)],
                                            ps[:], bqk_sb[:, 2 + t:3 + t])
                if dbg and t == 0 and sh == 1:
                    nc.sync.dma_start(dbg["dbg_kt0"].ap(), kt[0][:])
            else:
                nc.vector.tensor_scalar_add(
                    qz[2 * t][0:HD, HS * sh:HS * (sh + 1)],
                    ps[0:HD, :], bqk_sb[0:HD, t:t + 1])
                nc.vector.tensor_scalar_add(
                    qz[2 * t + 1][HD:P, HS * sh:HS * (sh + 1)],
                    ps[HD:P, :], bqk_sb[HD:P, t:t + 1])

        def vproj_pair(pr):
            ps = op_ps.tile([P, 2 * FL], F32, name=f"psv{pr}", tag="op")
            for half in range(2):
                st = 2 * pr + half
                for d in range(DCH):
                    nc.tensor.matmul(ps[:, FL * half:FL * (half + 1)],
                                     xt[d][:, P * st:P * (st + 1)], wv[d][:],
                                     start=(d == 0), stop=(d == DCH - 1))
            for half in range(2):
                st = 2 * pr + half
                v3 = vt[st][:, 0:HL * (HD + 1)].rearrange(
                    "p (h c) -> p h c", c=HD + 1)
                nc.vector.tensor_copy(
                    v3[:, :, 0:HD],
                    ps[:, FL * half:FL * (half + 1)].rearrange(
                        "p (h dd) -> p h dd", dd=HD))
            if dbg and pr == 0:
                nc.sync.dma_start(dbg["dbg_vt0"].ap(), vt[0][:])

        op_live = {}

        def run_op_half(st, phase, act_evac=False):
            if phase == 0:
                ops = [op_ps.tile([P, QC], F32, name=f"op{st}_{eb}", tag="op")
                       for eb in range(2)]
                op_live[st] = ops
                for eb in range(2):
                    nc.tensor.matmul(ops[eb][:],
                                     attn_sb[0][:, P * st:P * (st + 1)],
                                     wo[0][:, QC * eb:QC * (eb + 1)],
                                     start=True, stop=False)
                return
            ops = op_live.pop(st)
            for eb in range(2):
                nc.tensor.matmul(ops[eb][:],
                                 attn_sb[1][:, P * st:P * (st + 1)],
                                 wo[1][:, QC * eb:QC * (eb + 1)],
                                 start=False, stop=True)
            if act_evac:
                nc.scalar.activation(osb[st][:, 0:QC], ops[0][:], AF.Identity)
            else:
                nc.vector.tensor_copy(osb[st][:, 0:QC], ops[0][:])
            nc.vector.tensor_copy(osb[st][:, QC:2 * QC], ops[1][:])
            nc.sync.dma_start(out.ap()[P * st:P * (st + 1), :], osb[st][:])

        def normalize(qc, t, par, at):
            last = (qc == NQC - 1 and t == 1)
            atsb = small_pool.tile([HD, QC], F32,
                                   name=f"atsb{qc}_{t}_{par}", tag="atsb")
            zr = small_pool.tile([1, QC], F32, name=f"zr{qc}_{t}_{par}",
                                 tag="zr")
            if last:
                nc.scalar.activation(atsb[:], at[0:HD, :], AF.Identity)
                nc.scalar.activation(zr[:], at[HD:HD + 1, :], AF.Identity)
            else:
                nc.vector.tensor_copy(atsb[:], at[0:HD, :])
                nc.vector.tensor_copy(zr[:], at[HD:HD + 1, :])
            rz = small_pool.tile([1, QC], F32, name=f"rz{qc}_{t}_{par}",
                                 tag="rz")
            nc.vector.reciprocal_approx_fast(rz[:], zr[:])
            rzb = small_pool.tile([HD, QC], F32, name=f"rzb{qc}_{t}_{par}",
                                  tag="rzb")
            nc.gpsimd.partition_broadcast(rzb[:], rz[:])
            nc.vector.tensor_mul(
                attn_sb[t][HD * par:HD * (par + 1), QC * qc:QC * (qc + 1)],
                atsb[:], rzb[:])
            if dbg and qc == 0 and t == 0 and par == 0:
                nc.sync.dma_start(dbg["dbg_atsb"].ap(), atsb[:])
                nc.sync.dma_start(dbg["dbg_rz"].ap(), rz[:])
                nc.sync.dma_start(dbg["dbg_rzb"].ap(), rzb[:])

        units = [(qc, t, kk, par) for qc in range(NQC) for t in range(2)
                 for kk in range(NKK) for par in range(2)]
        # interjected work, keyed by unit index (deadline-driven):
        # kt[1]/qp[1] before unit 32, all vt pairs before their PV drains,
        # second-half q tiles before unit 128.
        interject = {
            0: [("k", 0, 1)], 2: [("v", 1)], 6: [("v", 2)],
            10: [("k", 1, 0)], 14: [("v", 3)], 18: [("k", 1, 1)],
            22: [("v", 4)], 26: [("q", 1, 0)], 30: [("v", 5)],
            34: [("v", 6)], 38: [("v", 7)], 48: [("q", 0, 1)],
            56: [("q", 1, 1)],
        }
        pend = []            # (qc, t, kk, par, group, e_tile, col_off)
        at_tiles = {}
        vpair_unit = {0: -100}
        op_queue = []
        group_idx = 0
        sc_cur, cur = None, []

        def drain_one(u, force=False):
            if not pend:
                return False
            qc, t, kk, par, g, et, off = pend[0]
            if not force:
                if g >= group_idx - (2 if kk == 0 else 0):
                    return False
                if vpair_unit.get(kk // 2, 10 ** 9) > u - 3:
                    return False
            pend.pop(0)
            key = (qc, t, par)
            if key not in at_tiles:
                at_tiles[key] = at_ps.tile([HD + 1, QC], F32,
                                           name=f"at{qc}_{t}_{par}", tag="at")
            h = 2 * t + par
            nc.tensor.matmul(at_tiles[key][:], vt[kk][:, 65 * h:65 * h + 65],
                             et[:, off:off + QC],
                             start=(kk == 0), stop=(kk == NKK - 1))
            if kk == NKK - 1:
                normalize(qc, t, par, at_tiles.pop(key))
                if t == 1 and par == 1:
                    op_queue.extend((st, ph, u + 16) for st in
                                    range(4 * qc, 4 * qc + 4)
                                    for ph in range(2))
            return True

        def proj_unit_fast(which, t):
            # sch-outer with split casts: the first 512 columns complete as
            # soon as the first DMA'd chunk of every xt tile lands
            ps = sc_ps.tile([P, HS], F32, name=f"psf{which}{t}", tag="sc")
            w = wk if which == "k" else wq
            for sch in range(2):
                for d in range(DCH):
                    nc.tensor.matmul(ps[:, QC * sch:QC * (sch + 1)],
                                     w[d][:, P * t:P * (t + 1)],
                                     xt[d][:, QC * sch:QC * (sch + 1)],
                                     start=(d == 0), stop=(d == DCH - 1))
                if which == "k":
                    nc.scalar.activation(
                        kt[t][:, QC * sch:QC * (sch + 1)],
                        ps[:, QC * sch:QC * (sch + 1)], AF.Identity,
                        bias=bqk_sb[:, 2 + t:3 + t])
                else:
                    nc.scalar.activation(
                        qz[2 * t][0:HD, QC * sch:QC * (sch + 1)],
                        ps[0:HD, QC * sch:QC * (sch + 1)], AF.Identity,
                        bias=bqk_sb[0:HD, t:t + 1])
                    nc.scalar.activation(
                        qz[2 * t + 1][HD:P, QC * sch:QC * (sch + 1)],
                        ps[HD:P, QC * sch:QC * (sch + 1)], AF.Identity,
                        bias=bqk_sb[HD:P, t:t + 1])

        # preamble: the minimum needed before the first score matmul
        proj_unit_fast("k", 0)
        proj_unit_fast("q", 0)
        vproj_pair(0)

        for u, (qc, t, kk, par) in enumerate(units):
            for ij in interject.get(u, []):
                if ij[0] == "v":
                    vproj_pair(ij[1])
                    vpair_unit[ij[1]] = u
                else:
                    proj_unit(*ij)
            if sc_cur is None:
                sc_cur = sc_ps.tile([P, 2 * QC], F32, name=f"sc{u}", tag="sc")
                cur = []
            nc.tensor.matmul(sc_cur[:, QC * len(cur):QC * (len(cur) + 1)],
                             kt[t][:, P * kk:P * (kk + 1)],
                             qz[2 * t + par][:, QC * qc:QC * (qc + 1)],
                             start=True, stop=True)
            cur.append((qc, t, kk, par))
            if len(cur) == 2:
                e = e_pool.tile([P, 2 * QC], BF16, name=f"e{u}", tag="e")
                nc.scalar.activation(e[:], sc_cur[:], AF.Exp)
                if dbg and u == 1:
                    nc.sync.dma_start(dbg["dbg_e"].ap(), e[:])
                for j, cu in enumerate(cur):
                    pend.append((*cu, group_idx, e, QC * j))
                group_idx += 1
                sc_cur = None
            drained = 0
            while drained < 4 and drain_one(u):
                drained += 1
            if u % 4 == 1 and op_queue and op_queue[0][2] <= u:
                st, ph, _ = op_queue.pop(0)
                run_op_half(st, ph)
        while pend:
            drain_one(10 ** 9, force=True)
        while op_queue:
            st, ph, _ = op_queue.pop(0)
            run_op_half(st, ph, act_evac=True)
        if dbg:
            nc.sync.dma_start(dbg["dbg_asb0"].ap(), attn_sb[0][:])

    ctx.close()


_CACHE = {}


def _get_program():
    if "nc" not in _CACHE:
        _CACHE["nc"] = build_program()
    return _CACHE["nc"]


def prep_inputs(input_tensor, qkv_weight, qkv_bias, out_weight, out_bias):
    """Host-side shard + transpose + cast. Returns in_maps for 8 cores."""
    x = np.asarray(input_tensor, np.float32)
    wqkv = np.asarray(qkv_weight, np.float32).copy()
    bqkv = np.asarray(qkv_bias, np.float32).copy()
    wout = np.asarray(out_weight, np.float32)
    scale = 1.0 / np.sqrt(np.float32(HD))
    wqkv[:D] *= scale
    bqkv[:D] *= scale
    bf = ml_dtypes.bfloat16
    woutT = np.ascontiguousarray(wout.T)
    xTb = [np.ascontiguousarray(x[b].T).astype(bf) for b in range(B)]
    in_maps = []
    for c in range(N_CORES):
        b, g = c // G, c % G
        lo = FL * g
        wqT = np.ascontiguousarray(wqkv[lo:lo + FL, :].T).astype(bf)
        wkT = np.ascontiguousarray(wqkv[D + lo:D + lo + FL, :].T).astype(bf)
        wvT = np.ascontiguousarray(
            wqkv[2 * D + lo:2 * D + lo + FL, :].T).astype(bf)
        woTg = np.ascontiguousarray(woutT[lo:lo + FL, :]).astype(bf)
        bq = bqkv[lo:lo + FL].reshape(2, P).T
        bk = bqkv[D + lo:D + lo + FL].reshape(2, P).T
        bqk = np.ascontiguousarray(
            np.concatenate([bq, bk], 1)).astype(np.float32)
        in_maps.append({"xT": xTb[b], "wqT": wqT, "wkT": wkT, "wvT": wvT,
                       "woT": woTg, "bqk": bqk})
    return in_maps


def assemble(outs, qkv_bias, out_weight, out_bias):
    """Sum the per-core partials and add the (V-bias-folded) output bias."""
    bqkv = np.asarray(qkv_bias, np.float32)
    wout = np.asarray(out_weight, np.float32)
    bout_eff = np.asarray(out_bias, np.float32) + wout @ bqkv[2 * D:]
    full = np.empty((B, S, D), np.float32)
    for b in range(B):
        acc = bout_eff[None, :].astype(np.float32).repeat(S, 0)
        for g in range(G):
            acc += np.asarray(outs[b * G + g], np.float32)
        full[b] = acc
    return full


def kernel(input_tensor, qkv_weight, qkv_bias, out_weight, out_bias,
           **run_kwargs):
    nc = _get_program()
    in_maps = prep_inputs(input_tensor, qkv_weight, qkv_bias, out_weight,
                          out_bias)
    res = run_bass_kernel_spmd(nc, in_maps, core_ids=list(range(N_CORES)),
                               **run_kwargs)
    full = assemble([res.results[c]["out"] for c in range(N_CORES)],
                    qkv_bias, out_weight, out_bias)
    if run_kwargs:
        kernel.last_results = res
    return full
